# revision 59
# baseline (speedup 1.0000x reference)
"""MoE feed-forward (8 experts, top-2) on 8 TRN2 NeuronCores, expert-parallel.

v5: locality-maximizing token sharding + per-ctile returns + warmup a2a.

The kernel receives the full inputs, so the host chooses the token->core
sharding itself: a small augmenting-path matching places (almost) every
token on a core that owns one of its two experts. Each core then computes
~1024 "own" rows straight from a host-pre-transposed input (no gathers, no
PE transposes, no network) before it touches the first remote row. That
~270us of local work hides the collective subsystem's ~120us cold-start
(absorbed by a tiny no-dependency warmup AllToAll queued first), and the
all-to-all traffic is half of a naive dispatch (only the second expert of
each token crosses the network). Dispatch is 4 chunked AllToAlls fed by a
gpsimd-only load+scatter cascade; results return in one small AllToAll per
compute ctile so every return lands well before its tokens are combined.
The combine waves carry explicit position deps so the compile-time
scheduler cannot hoist them ahead of ctile work (a v3 failure mode).
"""
import numpy as np
import ml_dtypes

import concourse.bass as bass
import concourse.mybir as mybir
import concourse.tile as tile
from concourse import bacc
from concourse.bass import IndirectOffsetOnAxis
from concourse.bass_utils import run_bass_kernel_spmd
from concourse.masks import make_identity

D_MODEL, HIDDEN, N_EXPERTS, TOP_K = 1024, 4096, 8, 2
N_CORES = 8
P = 128
T = 8192
T_LOC = T // N_CORES            # 1024 tokens per core
N_TOK_TILES = T_LOC // P        # 8
D_BLKS = D_MODEL // P           # 8
H_BLKS = HIDDEN // P            # 32
N_CT = 512                      # max token tile in expert-compute phase
OWN = T_LOC                     # own-expert row region: one row per token
N_OWN_CT = OWN // N_CT          # 2 own ctiles of 512
NCH = 4                         # dispatch chunks (2 token tiles each)

FP32 = mybir.dt.float32
BF16 = mybir.dt.bfloat16
I32 = mybir.dt.int32
AF = mybir.ActivationFunctionType
ALU = mybir.AluOpType
BF16_NP = ml_dtypes.bfloat16

RG = [list(range(N_CORES))]
OOB = 1 << 24                   # skipped by bounds_check on indirect DMA


def _dram_alias(nc, base_handle, name, shape=None):
    """A DRAM tensor handle aliasing base_handle's memory. Distinct names keep
    Tile's conservative same-tensor tracking from serializing writers that
    touch disjoint rows; readers declare deps explicitly."""
    if shape is None:
        shape = list(base_handle.shape)
    mls = nc._tensor(name, shape, base_handle.dtype,
                     kind="Internal", type="DRAM")
    base_mloc = nc.lookup_mloc(base_handle)
    mloc = mls.memorylocations[0]
    mloc.allocated = base_mloc.allocated
    mloc.addr = base_mloc.addr
    return bass.DRamTensorHandle(name, shape, base_handle.dtype)


def _ctile_sizes(s_all):
    """[512, 512] own ctiles, then 512-wide net ctiles ending in two
    128-row ctiles: the final return AllToAlls stay tiny and the
    second-to-last fires a full ctile earlier."""
    sizes = [N_CT] * N_OWN_CT
    rem = s_all - OWN
    while rem > 768:
        sizes.append(512)
        rem -= 512
    if rem > 256:
        sizes.append(rem - 256)
        rem = 256
    sizes.append(128)
    sizes.append(128)
    rem -= 256
    assert rem == 0
    out, off = [], 0
    for nt in sizes:
        out.append((off, nt))
        off += nt
    assert off == s_all
    return out


class Plan:
    """Per-input compile-time schedule (uniform across cores)."""

    def __init__(self, caps, s_all, cdep, crs, gmax,
                 p2_mask=(False,) * N_TOK_TILES):
        self.caps = list(caps)      # dispatch per-(src,dst) capacity per chunk
        self.s_all = s_all          # compute rows per core (mult of 128)
        self.cdep = list(cdep)      # per ctile: last dispatch chunk needed
        self.crs = list(crs)        # per return group (one per net ctile)
        self.gmax = tuple(gmax)     # per owner tile: last group it needs
        self.p2_mask = tuple(p2_mask)

        self.ctiles = _ctile_sizes(s_all)
        self.nc_tiles = len(self.ctiles)
        self.n_groups = self.nc_tiles - N_OWN_CT
        assert len(self.crs) == self.n_groups

        # send_x / recv_x layout: chunk regions then scratch (recv only)
        self.x_base = []
        off = 0
        for c in self.caps:
            self.x_base.append(off)
            off += N_CORES * c
        self.xs_rows = off
        self.xr_scratch = off
        self.xr_rows = off + P
        # send_y layout: [group regions][scratch]
        self.ys_base = []
        off = 0
        for cr in self.crs:
            self.ys_base.append(off)
            off += N_CORES * cr
        self.ys_scratch = off
        self.ys_rows = off + P
        # recv_y layout: [group regions][own results][scratch]
        self.yr_base = self.ys_base
        self.yr_own = self.ys_scratch
        self.yr_scratch = self.yr_own + OWN
        self.yr_rows = self.yr_scratch + P

    def key(self):
        return (tuple(self.caps), self.s_all, tuple(self.cdep),
                tuple(self.crs), self.gmax, self.p2_mask)


def _body(tc, plan, x_bf, xT_own, w1_loc, w2_loc, b1_t, b2_rep, rows_net,
          gidx_in, gout_t, gres_t, wts_t, gp2_t, op2_t, out_loc):
    nc = tc.nc
    p = plan
    NCOL = p.s_all // P
    nG = p.n_groups

    warm_in_t = nc.dram_tensor("warm_in", [N_CORES, D_MODEL], BF16)
    warm_out_t = nc.dram_tensor("warm_out", [N_CORES, D_MODEL], BF16)
    send_x_t = nc.dram_tensor("send_x", [p.xs_rows, D_MODEL], BF16)
    recv_x_t = nc.dram_tensor("recv_x", [p.xr_rows, D_MODEL], BF16)
    send_y_t = nc.dram_tensor("send_y", [p.ys_rows, D_MODEL], BF16)
    recv_y_t = nc.dram_tensor("recv_y", [p.yr_rows, D_MODEL], BF16)

    sxa = [_dram_alias(nc, send_x_t, f"sx_al{i}") for i in range(16)]
    send_x = send_x_t.ap()
    rxa = [_dram_alias(nc, recv_x_t, f"rx_al{h}") for h in range(NCH)]
    sya = [_dram_alias(nc, send_y_t, f"sy_al{i}") for i in range(NCOL)]
    rya = [_dram_alias(nc, recv_y_t, f"ry_al{i}")
           for i in range(OWN // P + nG)]

    recv_x = recv_x_t.ap()
    send_y = send_y_t.ap()
    recv_y = recv_y_t.ap()

    with tc.tile_pool(name="persist", bufs=1) as persist:
        ident_bf = persist.tile([P, P], BF16)
        make_identity(nc, ident_bf)
        w1_sb = persist.tile([P, D_BLKS, HIDDEN], BF16)
        w2_sb = persist.tile([P, H_BLKS, D_MODEL], BF16)
        b1_sb = persist.tile([P, H_BLKS], FP32)
        b2r_sb = persist.tile([P, D_MODEL], FP32)
        rnet_sb = persist.tile([P, N_TOK_TILES, TOP_K], I32)
        gin_sb = persist.tile([P, NCOL], I32)
        gout_sb = persist.tile([P, NCOL], I32)
        gres_sb = persist.tile([P, N_TOK_TILES], I32)
        wrow_sb = persist.tile([P, NCOL], FP32)
        gp2_sb = persist.tile([P, N_TOK_TILES, TOP_K], I32)
        op2_sb = persist.tile([P, N_TOK_TILES], I32)

        # rnet feeds the gpsimd dispatch scatters -> load it on that ring;
        # the scalar ring carries only b1 + the first own pieces so SILUs
        # are never stuck behind queued descriptors
        nc.gpsimd.dma_start(rnet_sb, rows_net[:])
        nc.scalar.dma_start(b1_sb, b1_t[:])

        with tc.tile_pool(name="phC", bufs=2) as pC, \
             tc.tile_pool(name="phE", bufs=1) as pE, \
             tc.tile_pool(name="phC_psum", bufs=3, space="PSUM") as pCp:

            # tiny no-dep warmup AllToAll: absorbs the collective
            # subsystem's cold-start latency so the real dispatch
            # AllToAlls begin promptly once triggered
            nc.gpsimd.collective_compute(
                "AllToAll", ALU.bypass, replica_groups=RG,
                ins=[warm_in_t.ap()[:, :].opt()],
                outs=[warm_out_t.ap()[:, :].opt()])


            # dispatch: per-chunk cascade entirely on the gpsimd ring —
            # load the chunk's two x tiles, scatter them into send_x
            # (SWDGE), trigger the chunk's AllToAll right away.
            cc_disp = []
            for h in range(NCH):
                scats = []
                for i in (2 * h, 2 * h + 1):
                    x_sb = pC.tile([P, D_MODEL], BF16, tag="x_sb",
                                   name="x_sb", bufs=2)
                    nc.gpsimd.dma_start(x_sb, x_bf[i * P:(i + 1) * P, :])
                    for k in range(TOP_K):
                        si = nc.gpsimd.indirect_dma_start(
                            out=sxa[i * TOP_K + k].ap(),
                            out_offset=IndirectOffsetOnAxis(
                                ap=rnet_sb[:, i, k:k + 1], axis=0),
                            in_=x_sb, in_offset=None,
                            bounds_check=p.xs_rows - 1, oob_is_err=False)
                        scats.append(si)
                lo = p.x_base[h]
                hi = lo + N_CORES * p.caps[h]
                cc = nc.gpsimd.collective_compute(
                    "AllToAll", ALU.bypass, replica_groups=RG,
                    ins=[send_x[lo:hi, :].opt()],
                    outs=[rxa[h].ap()[lo:hi, :].opt()])
                for si in scats:
                    bass._add_dep_helper(cc.ins, si.ins, sync=True,
                                         reason=f"a2a{h} after scatters")
                cc_disp.append(cc)

            # first own-ctile input: pieces split across the scalar and
            # sync rings ahead of the weights so all 8 land by ~15us
            xrTs = {}
            xrT0 = pC.tile([P, D_BLKS, N_CT], BF16, tag="xrT", name="xrT",
                           bufs=2)
            xrTs[0] = xrT0
            for j in range(D_BLKS):
                eng = nc.scalar if j < 4 else nc.sync
                eng.dma_start(xrT0[:, j, :], xT_own[:, 0, j, :])

            # both weight streams interleaved on the sync ring in
            # first-needed order
            W_CHUNK = 1024
            for hh in range(HIDDEN // W_CHUNK):
                for j in range(D_BLKS):
                    nc.sync.dma_start(
                        w1_sb[:, j, hh * W_CHUNK:(hh + 1) * W_CHUNK],
                        w1_loc[j * P:(j + 1) * P,
                               hh * W_CHUNK:(hh + 1) * W_CHUNK])
                for m in range(8 * hh, 8 * hh + 8):
                    nc.sync.dma_start(w2_sb[:, m, :],
                                      w2_loc[m * P:(m + 1) * P, :])

            xgs = {}

            def emit_own_load(ct, eng=None):
                """Own ctile input arrives pre-transposed from the host;
                per-j piece DMAs spread across engines (one big DMA
                trickles ~20us through a single queue)."""
                eng = eng or nc.scalar
                xrT = pC.tile([P, D_BLKS, N_CT], BF16, tag="xrT", name="xrT",
                              bufs=2)
                xrTs[ct] = xrT
                for j in range(D_BLKS):
                    eng.dma_start(xrT[:, j, :], xT_own[:, ct, j, :])

            def emit_gather(ct):
                """Indirect row gathers (SWDGE: unaffected by in-flight
                collectives) for net ctiles."""
                r0, NT = p.ctiles[ct]
                tiles = []
                for cc in range(NT // P):
                    col = r0 // P + cc
                    xg = pC.tile([P, D_MODEL], BF16, tag="xg", name="xg",
                                 bufs=4)
                    gi = nc.gpsimd.indirect_dma_start(
                        out=xg, out_offset=None, in_=recv_x[:],
                        in_offset=IndirectOffsetOnAxis(
                            ap=gin_sb[:, col:col + 1], axis=0),
                        bounds_check=p.xr_rows - 1, oob_is_err=False)
                    for h in range(p.cdep[ct] + 1):
                        bass._add_dep_helper(gi.ins, cc_disp[h].ins,
                                             sync=True,
                                             reason=f"gather after a2a{h}")
                    tiles.append(xg)
                xgs[ct] = tiles

            def emit_transpose(ct):
                """PE transposes (identity matmuls) feeding xrT, emitted just
                before the ctile's compute."""
                r0, NT = p.ctiles[ct]
                xrT = pC.tile([P, D_BLKS, N_CT], BF16, tag="xrT", name="xrT",
                              bufs=2)
                xrTs[ct] = xrT
                for cc in range(NT // P):
                    xg = xgs[ct][cc]
                    for j in range(D_BLKS):
                        tp = pCp.tile([P, P], BF16, tag="tp", name="tp",
                                      bufs=2)
                        nc.tensor.transpose(tp, xg[:, j * P:(j + 1) * P],
                                            ident_bf)
                        nc.vector.tensor_copy(
                            xrT[:, j, cc * P:(cc + 1) * P], tp)
                xgs.pop(ct)

            grp_scatters = [[] for _ in range(nG)]
            own_scatters = []
            last_scat = {}
            cc_rets = [None] * nG

            def emit_compute(ct, mid_cb=None):
                r0, NT = p.ctiles[ct]
                xrT = xrTs.pop(ct)
                hT = pC.tile([P, H_BLKS, N_CT], BF16, tag="hT", name="hT",
                             bufs=1)
                for m in range(H_BLKS):
                    if m == 6 and mid_cb is not None:
                        mid_cb()
                    ps1 = pCp.tile([P, N_CT], FP32, tag="ps1", name="ps1",
                                   bufs=3)
                    for j in range(D_BLKS):
                        nc.tensor.matmul(ps1[:, :NT],
                                         lhsT=w1_sb[:, j, m * P:(m + 1) * P],
                                         rhs=xrT[:, j, :NT],
                                         start=(j == 0),
                                         stop=(j == D_BLKS - 1))
                    nc.scalar.activation(hT[:, m, :NT], ps1[:, :NT], AF.Silu,
                                         bias=b1_sb[:, m:m + 1])

                for t in range(NT // P):
                    col = r0 // P + t
                    y_tm = pC.tile([P, D_MODEL], BF16, tag="y_tm",
                                   name="y_tm", bufs=3)
                    for nh in range(2):
                        ps2 = pCp.tile([P, 512], FP32, tag="ps2", name="ps2",
                                       bufs=3)
                        for m in range(H_BLKS):
                            nc.tensor.matmul(
                                ps2, lhsT=hT[:, m, t * P:(t + 1) * P],
                                rhs=w2_sb[:, m, nh * 512:(nh + 1) * 512],
                                start=(m == 0), stop=(m == H_BLKS - 1))
                        nc.vector.tensor_add(
                            y_tm[:, nh * 512:(nh + 1) * 512], ps2,
                            b2r_sb[:, nh * 512:(nh + 1) * 512])
                        nc.vector.tensor_scalar_mul(
                            y_tm[:, nh * 512:(nh + 1) * 512],
                            y_tm[:, nh * 512:(nh + 1) * 512],
                            wrow_sb[:, col:col + 1])
                    if ct < N_OWN_CT:
                        si = nc.gpsimd.indirect_dma_start(
                            out=rya[col].ap(),
                            out_offset=IndirectOffsetOnAxis(
                                ap=gout_sb[:, col:col + 1], axis=0),
                            in_=y_tm[:], in_offset=None,
                            bounds_check=p.yr_rows - 1, oob_is_err=False)
                        own_scatters.append(si)
                        last_scat[ct] = si
                    else:
                        si = nc.gpsimd.indirect_dma_start(
                            out=sya[col].ap(),
                            out_offset=IndirectOffsetOnAxis(
                                ap=gout_sb[:, col:col + 1], axis=0),
                            in_=y_tm[:], in_offset=None,
                            bounds_check=p.ys_rows - 1, oob_is_err=False)
                        grp_scatters[ct - N_OWN_CT].append(si)
                        last_scat[ct] = si

            def emit_ret(g):
                lo = p.ys_base[g]
                hi = lo + N_CORES * p.crs[g]
                cc = nc.gpsimd.collective_compute(
                    "AllToAll", ALU.bypass, replica_groups=RG,
                    ins=[send_y[lo:hi, :].opt()],
                    outs=[rya[OWN // P + g].ap()[lo:hi, :].opt()])
                for si in grp_scatters[g]:
                    bass._add_dep_helper(cc.ins, si.ins, sync=True,
                                         reason=f"ret a2a{g} after scatters")
                cc_rets[g] = cc

            def emit_E(tiles, pos_dep):
                """Combine pass for owner token tiles. The own-expert result
                rows are contiguous (own row index == local token index), so
                they arrive by direct DMA on the sync ring; only the net row
                needs an indirect gather with deps on exactly the return
                groups this tile's tokens ride. pos_dep pins the ops late
                enough in every engine stream that they can never block
                ctile work."""
                for i in tiles:
                    g0 = pC.tile([P, D_MODEL], BF16, tag="xg", name="g0",
                                 bufs=4)
                    di = nc.sync.dma_start(
                        g0, recv_y[p.yr_own + i * P:p.yr_own + (i + 1) * P,
                                   :])
                    g1 = pC.tile([P, D_MODEL], BF16, tag="xg", name="g1",
                                 bufs=4)
                    gi1 = nc.gpsimd.indirect_dma_start(
                        out=g1, out_offset=None, in_=recv_y[:],
                        in_offset=IndirectOffsetOnAxis(
                            ap=gres_sb[:, i:i + 1], axis=0),
                        bounds_check=p.yr_rows - 1, oob_is_err=False)
                    for gi in (di, gi1):
                        for si in own_scatters:
                            bass._add_dep_helper(gi.ins, si.ins, sync=True,
                                                 reason="E after own scat")
                        if pos_dep is not None:
                            bass._add_dep_helper(gi.ins, pos_dep.ins,
                                                 sync=True,
                                                 reason="E position pin")
                    for g in range(p.gmax[i] + 1):
                        bass._add_dep_helper(gi1.ins, cc_rets[g].ins,
                                             sync=True,
                                             reason="E after ret a2a")
                    acc = pE.tile([P, D_MODEL], FP32, tag="acc", name="acc",
                                  bufs=2)
                    nc.vector.tensor_add(acc, g0, g1)
                    nc.sync.dma_start(out_loc[i * P:(i + 1) * P, :], acc)

            def emit_patch(lastg, pos_dep):
                # re-gather + re-combine + partial-store only the tokens
                # whose result rows live in the final return group
                for ii in range(N_TOK_TILES):
                    if not p.p2_mask[ii]:
                        continue
                    g0 = pC.tile([P, D_MODEL], BF16, tag="xg", name="p2g0",
                                 bufs=4)
                    gi0 = nc.gpsimd.indirect_dma_start(
                        out=g0, out_offset=None, in_=recv_y[:],
                        in_offset=IndirectOffsetOnAxis(
                            ap=gp2_sb[:, ii, 0:1], axis=0),
                        bounds_check=p.yr_rows - 1, oob_is_err=False)
                    g1 = pC.tile([P, D_MODEL], BF16, tag="xg", name="p2g1",
                                 bufs=4)
                    gi1 = nc.gpsimd.indirect_dma_start(
                        out=g1, out_offset=None, in_=recv_y[:],
                        in_offset=IndirectOffsetOnAxis(
                            ap=gp2_sb[:, ii, 1:2], axis=0),
                        bounds_check=p.yr_rows - 1, oob_is_err=False)
                    for gi in (gi0, gi1):
                        bass._add_dep_helper(gi.ins, cc_rets[lastg].ins,
                                             sync=True,
                                             reason="E p2 after last ret")
                        if pos_dep is not None:
                            bass._add_dep_helper(gi.ins, pos_dep.ins,
                                                 sync=True,
                                                 reason="E p2 position pin")
                    acc = pE.tile([P, D_MODEL], FP32, tag="acc", name="acc",
                                  bufs=2)
                    nc.vector.tensor_add(acc, g0, g1)
                    nc.gpsimd.indirect_dma_start(
                        out=out_loc,
                        out_offset=IndirectOffsetOnAxis(
                            ap=op2_sb[:, ii:ii + 1], axis=0),
                        in_=acc, in_offset=None,
                        bounds_check=T_LOC - 1, oob_is_err=False)

            def mid_tables():
                # tables needed from mid-ctile-0 onward: issued on the
                # gpsimd ring (idle once the dispatch cascade is out) so
                # they never delay the SILUs on the scalar ring
                nc.gpsimd.dma_start(gout_sb, gout_t[:])
                nc.gpsimd.dma_start(b2r_sb, b2_rep[:])
                emit_own_load(1, eng=nc.gpsimd)
                nc.gpsimd.dma_start(gin_sb, gidx_in[:])
                nc.gpsimd.dma_start(gres_sb, gres_t[:])
                nc.gpsimd.dma_start(wrow_sb, wts_t[:])
                nc.gpsimd.dma_start(gp2_sb, gp2_t[:])
                nc.gpsimd.dma_start(op2_sb, op2_t[:])

            # per-tile combine positions: one ctile after the tile's last
            # return group is triggered, so a slow return can never block
            # a later ctile's scatters through the gpsimd FIFO
            pw = [N_OWN_CT + p.gmax[i] + 1 for i in range(N_TOK_TILES)]
            io_done = N_OWN_CT
            for ct in range(p.nc_tiles):
                if ct >= N_OWN_CT:
                    emit_transpose(ct)
                emit_compute(ct, mid_cb=mid_tables if ct == 0 else None)
                if ct >= N_OWN_CT:
                    emit_ret(ct - N_OWN_CT)
                while io_done < p.nc_tiles and io_done <= ct + 2:
                    emit_gather(io_done)
                    io_done += 1
                tiles_now = [i for i in range(N_TOK_TILES) if pw[i] == ct]
                if tiles_now:
                    emit_E(tiles_now, pos_dep=last_scat[ct])
            lastc = p.nc_tiles - 1
            late = [i for i in range(N_TOK_TILES) if pw[i] > lastc]
            if late:
                emit_E(late, pos_dep=last_scat[lastc])
            emit_patch(nG - 1, pos_dep=last_scat[lastc])


def build_kernel(plan):
    nc = bacc.Bacc("TRN2", target_bir_lowering=False, debug=False,
                   num_devices=N_CORES)
    NCOL = plan.s_all // P
    args = dict(
        x_bf=nc.dram_tensor("x_bf", [T_LOC, D_MODEL], BF16,
                            kind="ExternalInput"),
        xT_own=nc.dram_tensor("xT_own", [P, N_OWN_CT, D_BLKS, N_CT], BF16,
                              kind="ExternalInput"),
        rows_net=nc.dram_tensor("rows_net", [P, N_TOK_TILES, TOP_K], I32,
                                kind="ExternalInput"),
        w1_loc=nc.dram_tensor("w1_loc", [D_MODEL, HIDDEN], BF16,
                              kind="ExternalInput"),
        w2_loc=nc.dram_tensor("w2_loc", [HIDDEN, D_MODEL], BF16,
                              kind="ExternalInput"),
        b1_t=nc.dram_tensor("b1_t", [P, H_BLKS], FP32, kind="ExternalInput"),
        b2_rep=nc.dram_tensor("b2_rep", [P, D_MODEL], FP32,
                              kind="ExternalInput"),
        gidx_in=nc.dram_tensor("gidx_in", [P, NCOL], I32,
                               kind="ExternalInput"),
        gout_t=nc.dram_tensor("gout_t", [P, NCOL], I32,
                              kind="ExternalInput"),
        gres_t=nc.dram_tensor("gres_t", [P, N_TOK_TILES], I32,
                              kind="ExternalInput"),
        wts_t=nc.dram_tensor("wts_t", [P, NCOL], FP32,
                             kind="ExternalInput"),
        gp2_t=nc.dram_tensor("gp2_t", [P, N_TOK_TILES, TOP_K], I32,
                             kind="ExternalInput"),
        op2_t=nc.dram_tensor("op2_t", [P, N_TOK_TILES], I32,
                             kind="ExternalInput"),
        out_loc=nc.dram_tensor("out_loc", [T_LOC, D_MODEL], FP32,
                               kind="ExternalOutput"),
    )
    with tile.TileContext(nc) as tc:
        _body(tc, plan, **{k: v.ap() for k, v in args.items()})
    nc.compile()
    return nc


def _round_up(v, m):
    return ((v + m - 1) // m) * m


def _assign_tokens(top2):
    """Token->core matching: place each token on a core owning one of its
    two experts (augmenting paths fix greedy failures; leftovers spill to
    any core with space)."""
    cap = np.full(N_CORES, T_LOC, np.int64)
    assign = np.full(T, -1, np.int64)
    members = [[] for _ in range(N_CORES)]   # tokens assigned per core
    flow = np.zeros((N_CORES, N_CORES), np.int64)   # src -> net dst counts

    def place(t, c):
        assign[t] = c
        members[c].append(t)
        cap[c] -= 1
        for e in top2[t]:
            if int(e) != c:
                flow[c, int(e)] += 1

    deferred = []
    for t in range(T):
        # prefer the expert-core that keeps the directed pair flows
        # balanced (smaller send-count for this pair orientation)
        opts = [int(top2[t, k]) for k in range(TOP_K)]
        a, b = opts[0], opts[1]
        order_ = opts if flow[a, b] <= flow[b, a] else opts[::-1]
        for c in order_:
            if cap[c] > 0:
                place(t, c)
                break
        else:
            deferred.append(t)

    spills = []
    for t in deferred:
        # augment: find u on a full core c in top2(t) that can move to its
        # other expert's core with free capacity
        done = False
        for k in range(TOP_K):
            c = int(top2[t, k])
            for u in members[c]:
                alts = [int(e) for e in top2[u] if e != c]
                for d in alts:
                    if cap[d] > 0:
                        members[c].remove(u)
                        assign[u] = -1
                        cap[c] += 1
                        for e in top2[u]:
                            if int(e) != c:
                                flow[c, int(e)] -= 1
                        place(u, d)
                        place(t, c)
                        done = True
                        break
                if done:
                    break
            if done:
                break
        if not done:
            spills.append(t)
    for t in spills:
        c = int(np.argmax(cap))
        place(t, c)
    assert (cap == 0).all() and (assign >= 0).all()
    # interleave each core's tokens round-robin by net destination so the
    # (src,dst) flow counts are uniform across dispatch chunks and owner
    # counts are uniform across return groups
    perm = []
    for c in range(N_CORES):
        buckets = [[] for _ in range(N_CORES)]
        for t in sorted(members[c]):
            dsts = [int(e) for e in top2[t] if e != c]
            buckets[dsts[0]].append(t)
        keyed = []
        for bi, b in enumerate(buckets):
            for k, t in enumerate(b):
                keyed.append(((k + 0.5) / len(b), bi, t))
        order = [t for _, _, t in sorted(keyed)]
        perm.append(np.array(order, np.int64))
    return perm


def _make_plan_and_tables(flat_x, gate_w, gate_b):
    """Exact host routing + schedule. Returns (plan, per-core tables, perm)."""
    x64 = flat_x.astype(np.float64)
    logits = x64 @ gate_w.astype(np.float64) + gate_b.astype(np.float64)
    order = np.argsort(-logits, axis=1, kind="stable")
    top2 = order[:, :TOP_K]                       # [T, 2]
    l0 = np.take_along_axis(logits, top2, axis=1)
    w0 = 1.0 / (1.0 + np.exp(-(l0[:, 0] - l0[:, 1])))
    wts = np.stack([w0, 1.0 - w0], axis=1).astype(np.float32)  # [T, 2]

    perm = _assign_tokens(top2)
    # per-core views in local token order
    core_of = np.empty(T, np.int64)
    loc_of = np.empty(T, np.int64)
    for c in range(N_CORES):
        core_of[perm[c]] = c
        loc_of[perm[c]] = np.arange(T_LOC)

    # own rows: the (at most one) expert matching the assigned core.
    # own row index == local token index.
    own_k = np.full(T, -1, np.int64)
    for t in range(T):
        for k in range(TOP_K):
            if top2[t, k] == core_of[t]:
                own_k[t] = k
                break

    # --- network slots per dispatch chunk (chunk = local tile pair) ---
    cnt_net = np.zeros((NCH, N_CORES, N_CORES), np.int64)  # [chunk, src, dst]
    net_slot = np.full((T, TOP_K), -1, np.int64)
    for t in range(T):
        s = core_of[t]
        h = loc_of[t] // (P * 2)
        for k in range(TOP_K):
            if k == own_k[t]:
                continue
            d = int(top2[t, k])
            net_slot[t, k] = cnt_net[h, s, d]
            cnt_net[h, s, d] += 1
    caps = [int(_round_up(max(1, cnt_net[h].max()), 16)) for h in range(NCH)]

    # --- per-core compute rows ---
    n_net_rows = cnt_net.sum(axis=1)              # [chunk, dst]
    per_core_rows = OWN + n_net_rows.sum(axis=0)
    s_all = int(_round_up(per_core_rows.max(), P))

    ctl = _ctile_sizes(s_all)
    ends = [off + nt for off, nt in ctl]
    n_ctiles = len(ctl)
    nG = n_ctiles - N_OWN_CT
    avail = [OWN + int(n_net_rows[:h + 1].sum(axis=0).min())
             for h in range(NCH)]
    cdep = []
    for ct, e in enumerate(ends):
        if e <= OWN:
            cdep.append(-1)
        else:
            cdep.append(next((h for h in range(NCH) if e <= avail[h]),
                             NCH - 1))

    row_to_ct = np.zeros(s_all, np.int64)
    for ct, (off, nt) in enumerate(ctl):
        row_to_ct[off:off + nt] = ct

    probe = Plan(caps, s_all, cdep, [16] * nG, (0,) * N_TOK_TILES)
    XNET = probe.x_base
    XSCR = probe.xr_scratch

    gin = np.zeros((N_CORES, s_all), np.int64)
    row_kind = np.zeros((N_CORES, s_all), np.int8)   # 0 pad, 1 own, 2 net
    row_owner = np.zeros((N_CORES, s_all), np.int64)
    net_comp_row = np.zeros((NCH, N_CORES, N_CORES, max(caps)), np.int64)
    for c in range(N_CORES):
        row_kind[c, :OWN] = 1
        row_owner[c, :OWN] = c
        r = OWN
        for h in range(NCH):
            # round-robin across srcs: every ctile's result rows spread
            # evenly over owners -> small per-group return capacities
            hi = int(cnt_net[h, :, c].max())
            for i in range(hi):
                for s in range(N_CORES):
                    if i < int(cnt_net[h, s, c]):
                        gin[c, r] = XNET[h] + s * caps[h] + i
                        row_kind[c, r] = 2
                        row_owner[c, r] = s
                        net_comp_row[h, s, c, i] = r
                        r += 1
        while r < s_all:
            gin[c, r] = XSCR + (r % P)
            r += 1

    # per-compute-row routing weights (applied expert-side to y rows)
    wrow = np.zeros((N_CORES, s_all), np.float32)
    for t in range(T):
        s = core_of[t]
        for k in range(TOP_K):
            if k == own_k[t]:
                wrow[s][loc_of[t]] = wts[t, k]
            else:
                d = int(top2[t, k])
                h = loc_of[t] // (P * 2)
                r = int(net_comp_row[h, s, d, int(net_slot[t, k])])
                wrow[d][r] = wts[t, k]

    # per-ctile return groups
    crs_count = np.zeros((N_CORES, nG, N_CORES), np.int64)
    ret_idx = np.zeros((N_CORES, s_all), np.int64)
    ret_grp = np.full((N_CORES, s_all), -1, np.int64)
    for c in range(N_CORES):
        for r in range(OWN, s_all):
            if row_kind[c, r] != 2:
                continue
            g = int(row_to_ct[r]) - N_OWN_CT
            o = row_owner[c, r]
            ret_grp[c, r] = g
            ret_idx[c, r] = crs_count[c, g, o]
            crs_count[c, g, o] += 1
    crs = [int(_round_up(max(1, crs_count[:, g, :].max()), 8))
           for g in range(nG)]

    plan = Plan(caps, s_all, cdep, crs, (0,) * N_TOK_TILES)

    # --- device tables per core ---
    NCOL = s_all // P
    tabs = []
    lastg = nG - 1
    p2_any = np.zeros(N_TOK_TILES, bool)
    gmax = np.zeros(N_TOK_TILES, np.int64)
    x_bf_full = flat_x.astype(BF16_NP)
    for c in range(N_CORES):
        x_core = np.ascontiguousarray(x_bf_full[perm[c]])   # [T_LOC, D]
        rows_net_t = np.full((P, N_TOK_TILES, TOP_K), OOB, np.int32)
        gres = np.zeros((P, N_TOK_TILES), np.int32)
        gp2 = np.full((P, N_TOK_TILES, TOP_K), OOB, np.int32)
        op2 = np.full((P, N_TOK_TILES), OOB, np.int32)
        for tl in range(T_LOC):
            t = int(perm[c][tl])
            i, pp = tl // P, tl % P
            h = tl // (P * 2)
            nets = []
            for k in range(TOP_K):
                if k == own_k[t]:
                    continue
                d = int(top2[t, k])
                idx = int(net_slot[t, k])
                rows_net_t[pp, i, k] = XNET[h] + d * caps[h] + idx
                r = int(net_comp_row[h, c, d, idx])
                g = int(ret_grp[d, r])
                nets.append((plan.yr_base[g] + d * crs[g]
                             + int(ret_idx[d, r]), g))
            gres[pp, i] = nets[0][0]
            gmax[i] = max(gmax[i], nets[0][1])
            if len(nets) > 1:
                # spilled token (no own row): the tile's combine adds a
                # zero own row + nets[0]; the patch rewrites with both
                # net rows after the final return
                p2_any[i] = True
                gp2[pp, i, 0] = nets[0][0]
                gp2[pp, i, 1] = nets[1][0]
                op2[pp, i] = tl

        gout = np.zeros((P, NCOL), np.int32)
        for r in range(s_all):
            col, pp = r // P, r % P
            if row_kind[c, r] == 1:
                gout[pp, col] = plan.yr_own + r   # own row j == r
            elif row_kind[c, r] == 2:
                g = int(ret_grp[c, r])
                o = int(row_owner[c, r])
                gout[pp, col] = (plan.ys_base[g] + o * crs[g]
                                 + int(ret_idx[c, r]))
            else:
                gout[pp, col] = plan.ys_scratch + pp
        gin32 = np.ascontiguousarray(
            gin[c].reshape(NCOL, P).T.astype(np.int32))
        gout = np.ascontiguousarray(gout)
        wrow32 = np.ascontiguousarray(wrow[c].reshape(NCOL, P).T)

        # whole per-core x pre-transposed, packed per own-ctile
        xT_own = np.zeros((P, N_OWN_CT, D_BLKS, N_CT), BF16_NP)
        A = x_core.reshape(N_OWN_CT, N_CT, D_BLKS, P)
        xT_own[:, :, :, :] = A.transpose(3, 0, 2, 1)

        tabs.append(dict(gidx_in=gin32, gout_t=gout, gres_t=gres,
                         wts_t=wrow32, gp2_t=gp2, op2_t=op2,
                         rows_net=rows_net_t,
                         x_bf=x_core,
                         xT_own=np.ascontiguousarray(xT_own)))
    plan.p2_mask = tuple(bool(x) for x in p2_any)
    plan.gmax = tuple(int(g) for g in gmax)
    return plan, tabs, perm


_CACHE = {}


def kernel(x, gate_w, gate_b, w1, b1, w2, b2, _trace=False):
    x = np.ascontiguousarray(np.asarray(x, dtype=np.float32))
    gate_w = np.ascontiguousarray(np.asarray(gate_w, dtype=np.float32))
    gate_b = np.ascontiguousarray(np.asarray(gate_b, dtype=np.float32))
    w1 = np.ascontiguousarray(np.asarray(w1, dtype=np.float32))
    b1 = np.ascontiguousarray(np.asarray(b1, dtype=np.float32))
    w2 = np.ascontiguousarray(np.asarray(w2, dtype=np.float32))
    b2 = np.ascontiguousarray(np.asarray(b2, dtype=np.float32))

    orig_shape = x.shape
    flat_x = x.reshape(-1, D_MODEL)
    plan, tabs, perm = _make_plan_and_tables(flat_x, gate_w, gate_b)

    if plan.key() not in _CACHE:
        _CACHE[plan.key()] = build_kernel(plan)
    nc = _CACHE[plan.key()]

    in_maps = []
    for c in range(N_CORES):
        m = dict(tabs[c])
        m["w1_loc"] = np.ascontiguousarray(w1[c].astype(BF16_NP))
        m["w2_loc"] = np.ascontiguousarray(w2[c].astype(BF16_NP))
        m["b1_t"] = np.ascontiguousarray(b1[c].reshape(H_BLKS, P).T)
        m["b2_rep"] = np.tile(b2[c], (P, 1))
        in_maps.append(m)

    res = run_bass_kernel_spmd(nc, in_maps, core_ids=list(range(N_CORES)),
                               trace=_trace)
    out = np.empty((T, D_MODEL), np.float32)
    for c in range(N_CORES):
        out[perm[c]] = res.results[c]["out_loc"]
    if _trace:
        kernel.last_results = res
    return out.reshape(orig_shape)


# revision 60
# speedup vs baseline: 1.0017x; 1.0017x over previous
"""MoE feed-forward (8 experts, top-2) on 8 TRN2 NeuronCores, expert-parallel.

v5: locality-maximizing token sharding + per-ctile returns + warmup a2a.

The kernel receives the full inputs, so the host chooses the token->core
sharding itself: a small augmenting-path matching places (almost) every
token on a core that owns one of its two experts. Each core then computes
~1024 "own" rows straight from a host-pre-transposed input (no gathers, no
PE transposes, no network) before it touches the first remote row. That
~270us of local work hides the collective subsystem's ~120us cold-start
(absorbed by a tiny no-dependency warmup AllToAll queued first), and the
all-to-all traffic is half of a naive dispatch (only the second expert of
each token crosses the network). Dispatch is 4 chunked AllToAlls fed by a
gpsimd-only load+scatter cascade; results return in one small AllToAll per
compute ctile so every return lands well before its tokens are combined.
The combine waves carry explicit position deps so the compile-time
scheduler cannot hoist them ahead of ctile work (a v3 failure mode).
"""
import numpy as np
import ml_dtypes

import concourse.bass as bass
import concourse.mybir as mybir
import concourse.tile as tile
from concourse import bacc
from concourse.bass import IndirectOffsetOnAxis
from concourse.bass_utils import run_bass_kernel_spmd
from concourse.masks import make_identity

D_MODEL, HIDDEN, N_EXPERTS, TOP_K = 1024, 4096, 8, 2
N_CORES = 8
P = 128
T = 8192
T_LOC = T // N_CORES            # 1024 tokens per core
N_TOK_TILES = T_LOC // P        # 8
D_BLKS = D_MODEL // P           # 8
H_BLKS = HIDDEN // P            # 32
N_CT = 512                      # max token tile in expert-compute phase
OWN = T_LOC                     # own-expert row region: one row per token
N_OWN_CT = OWN // N_CT          # 2 own ctiles of 512
NCH = 4                         # dispatch chunks (2 token tiles each)

FP32 = mybir.dt.float32
BF16 = mybir.dt.bfloat16
I32 = mybir.dt.int32
AF = mybir.ActivationFunctionType
ALU = mybir.AluOpType
BF16_NP = ml_dtypes.bfloat16

RG = [list(range(N_CORES))]
OOB = 1 << 24                   # skipped by bounds_check on indirect DMA


def _dram_alias(nc, base_handle, name, shape=None):
    """A DRAM tensor handle aliasing base_handle's memory. Distinct names keep
    Tile's conservative same-tensor tracking from serializing writers that
    touch disjoint rows; readers declare deps explicitly."""
    if shape is None:
        shape = list(base_handle.shape)
    mls = nc._tensor(name, shape, base_handle.dtype,
                     kind="Internal", type="DRAM")
    base_mloc = nc.lookup_mloc(base_handle)
    mloc = mls.memorylocations[0]
    mloc.allocated = base_mloc.allocated
    mloc.addr = base_mloc.addr
    return bass.DRamTensorHandle(name, shape, base_handle.dtype)


def _ctile_sizes(s_all):
    """[512, 512] own ctiles, then 512-wide net ctiles ending in two
    128-row ctiles: the final return AllToAlls stay tiny and the
    second-to-last fires a full ctile earlier."""
    sizes = [N_CT] * N_OWN_CT
    rem = s_all - OWN
    while rem > 768:
        sizes.append(512)
        rem -= 512
    if rem > 256:
        sizes.append(rem - 256)
        rem = 256
    sizes.append(128)
    sizes.append(128)
    rem -= 256
    assert rem == 0
    out, off = [], 0
    for nt in sizes:
        out.append((off, nt))
        off += nt
    assert off == s_all
    return out


class Plan:
    """Per-input compile-time schedule (uniform across cores)."""

    def __init__(self, caps, s_all, cdep, crs, gmax,
                 p2_mask=(False,) * N_TOK_TILES):
        self.caps = list(caps)      # dispatch per-(src,dst) capacity per chunk
        self.s_all = s_all          # compute rows per core (mult of 128)
        self.cdep = list(cdep)      # per ctile: last dispatch chunk needed
        self.crs = list(crs)        # per return group (one per net ctile)
        self.gmax = tuple(gmax)     # per owner tile: last group it needs
        self.p2_mask = tuple(p2_mask)

        self.ctiles = _ctile_sizes(s_all)
        self.nc_tiles = len(self.ctiles)
        self.n_groups = self.nc_tiles - N_OWN_CT
        assert len(self.crs) == self.n_groups

        # send_x / recv_x layout: chunk regions then scratch (recv only)
        self.x_base = []
        off = 0
        for c in self.caps:
            self.x_base.append(off)
            off += N_CORES * c
        self.xs_rows = off
        self.xr_scratch = off
        self.xr_rows = off + P
        # send_y layout: [group regions][scratch]
        self.ys_base = []
        off = 0
        for cr in self.crs:
            self.ys_base.append(off)
            off += N_CORES * cr
        self.ys_scratch = off
        self.ys_rows = off + P
        # recv_y layout: [group regions][own results][scratch]
        self.yr_base = self.ys_base
        self.yr_own = self.ys_scratch
        self.yr_scratch = self.yr_own + OWN
        self.yr_rows = self.yr_scratch + P

    def key(self):
        return (tuple(self.caps), self.s_all, tuple(self.cdep),
                tuple(self.crs), self.gmax, self.p2_mask)


def _body(tc, plan, x_bf, xT_own, w1_loc, w2_loc, b1_t, b2_rep, rows_net,
          gidx_in, gout_t, gres_t, wts_t, gp2_t, op2_t, out_loc):
    nc = tc.nc
    p = plan
    NCOL = p.s_all // P
    nG = p.n_groups

    warm_in_t = nc.dram_tensor("warm_in", [N_CORES, D_MODEL], BF16)
    warm_out_t = nc.dram_tensor("warm_out", [N_CORES, D_MODEL], BF16)
    send_x_t = nc.dram_tensor("send_x", [p.xs_rows, D_MODEL], BF16)
    recv_x_t = nc.dram_tensor("recv_x", [p.xr_rows, D_MODEL], BF16)
    send_y_t = nc.dram_tensor("send_y", [p.ys_rows, D_MODEL], BF16)
    recv_y_t = nc.dram_tensor("recv_y", [p.yr_rows, D_MODEL], BF16)

    sxa = [_dram_alias(nc, send_x_t, f"sx_al{i}") for i in range(16)]
    send_x = send_x_t.ap()
    rxa = [_dram_alias(nc, recv_x_t, f"rx_al{h}") for h in range(NCH)]
    sya = [_dram_alias(nc, send_y_t, f"sy_al{i}") for i in range(NCOL)]
    rya = [_dram_alias(nc, recv_y_t, f"ry_al{i}")
           for i in range(OWN // P + nG)]

    recv_x = recv_x_t.ap()
    send_y = send_y_t.ap()
    recv_y = recv_y_t.ap()

    with tc.tile_pool(name="persist", bufs=1) as persist:
        ident_bf = persist.tile([P, P], BF16)
        make_identity(nc, ident_bf)
        w1_sb = persist.tile([P, D_BLKS, HIDDEN], BF16)
        w2_sb = persist.tile([P, H_BLKS, D_MODEL], BF16)
        b1_sb = persist.tile([P, H_BLKS], FP32)
        b2r_sb = persist.tile([P, D_MODEL], FP32)
        rnet_sb = persist.tile([P, N_TOK_TILES, TOP_K], I32)
        gin_sb = persist.tile([P, NCOL], I32)
        gout_sb = persist.tile([P, NCOL], I32)
        gres_sb = persist.tile([P, N_TOK_TILES], I32)
        wrow_sb = persist.tile([P, NCOL], FP32)
        gp2_sb = persist.tile([P, N_TOK_TILES, TOP_K], I32)
        op2_sb = persist.tile([P, N_TOK_TILES], I32)

        # rnet feeds the gpsimd dispatch scatters -> load it on that ring;
        # the scalar ring carries only b1 + the first own pieces so SILUs
        # are never stuck behind queued descriptors
        nc.gpsimd.dma_start(rnet_sb, rows_net[:])
        nc.scalar.dma_start(b1_sb, b1_t[:])

        with tc.tile_pool(name="phC", bufs=2) as pC, \
             tc.tile_pool(name="phE", bufs=1) as pE, \
             tc.tile_pool(name="phC_psum", bufs=3, space="PSUM") as pCp:

            # tiny no-dep warmup AllToAll: absorbs the collective
            # subsystem's cold-start latency so the real dispatch
            # AllToAlls begin promptly once triggered
            nc.gpsimd.collective_compute(
                "AllToAll", ALU.bypass, replica_groups=RG,
                ins=[warm_in_t.ap()[:, :].opt()],
                outs=[warm_out_t.ap()[:, :].opt()])


            # dispatch: per-chunk cascade entirely on the gpsimd ring —
            # load the chunk's two x tiles, scatter them into send_x
            # (SWDGE), trigger the chunk's AllToAll right away.
            cc_disp = []
            for h in range(NCH):
                scats = []
                for i in (2 * h, 2 * h + 1):
                    x_sb = pC.tile([P, D_MODEL], BF16, tag="x_sb",
                                   name="x_sb", bufs=2)
                    nc.gpsimd.dma_start(x_sb, x_bf[i * P:(i + 1) * P, :])
                    for k in range(TOP_K):
                        si = nc.gpsimd.indirect_dma_start(
                            out=sxa[i * TOP_K + k].ap(),
                            out_offset=IndirectOffsetOnAxis(
                                ap=rnet_sb[:, i, k:k + 1], axis=0),
                            in_=x_sb, in_offset=None,
                            bounds_check=p.xs_rows - 1, oob_is_err=False)
                        scats.append(si)
                lo = p.x_base[h]
                hi = lo + N_CORES * p.caps[h]
                cc = nc.gpsimd.collective_compute(
                    "AllToAll", ALU.bypass, replica_groups=RG,
                    ins=[send_x[lo:hi, :].opt()],
                    outs=[rxa[h].ap()[lo:hi, :].opt()])
                for si in scats:
                    bass._add_dep_helper(cc.ins, si.ins, sync=True,
                                         reason=f"a2a{h} after scatters")
                cc_disp.append(cc)

            # first own-ctile input: pieces split across the scalar and
            # sync rings ahead of the weights so all 8 land by ~15us
            xrTs = {}
            xrT0 = pC.tile([P, D_BLKS, N_CT], BF16, tag="xrT", name="xrT",
                           bufs=2)
            xrTs[0] = xrT0
            for j in range(D_BLKS):
                eng = nc.scalar if j < 4 else nc.sync
                eng.dma_start(xrT0[:, j, :], xT_own[:, 0, j, :])

            # weight streams on the sync ring in first-needed order:
            # ctile 0's mm1 consumes w1 chunks from ~16us but w2 only
            # from ~90us, so w1's first half streams before any w2
            W_CHUNK = 1024

            def w1_chunk(hh):
                for j in range(D_BLKS):
                    nc.sync.dma_start(
                        w1_sb[:, j, hh * W_CHUNK:(hh + 1) * W_CHUNK],
                        w1_loc[j * P:(j + 1) * P,
                               hh * W_CHUNK:(hh + 1) * W_CHUNK])

            w1_chunk(0)
            w1_chunk(1)
            for m in range(0, 8):
                nc.sync.dma_start(w2_sb[:, m, :],
                                  w2_loc[m * P:(m + 1) * P, :])
            w1_chunk(2)
            for m in range(8, 16):
                nc.sync.dma_start(w2_sb[:, m, :],
                                  w2_loc[m * P:(m + 1) * P, :])
            w1_chunk(3)
            for m in range(16, H_BLKS):
                nc.sync.dma_start(w2_sb[:, m, :],
                                  w2_loc[m * P:(m + 1) * P, :])

            xgs = {}

            def emit_own_load(ct, eng=None):
                """Own ctile input arrives pre-transposed from the host;
                per-j piece DMAs spread across engines (one big DMA
                trickles ~20us through a single queue)."""
                eng = eng or nc.scalar
                xrT = pC.tile([P, D_BLKS, N_CT], BF16, tag="xrT", name="xrT",
                              bufs=2)
                xrTs[ct] = xrT
                for j in range(D_BLKS):
                    eng.dma_start(xrT[:, j, :], xT_own[:, ct, j, :])

            def emit_gather(ct):
                """Indirect row gathers (SWDGE: unaffected by in-flight
                collectives) for net ctiles."""
                r0, NT = p.ctiles[ct]
                tiles = []
                for cc in range(NT // P):
                    col = r0 // P + cc
                    xg = pC.tile([P, D_MODEL], BF16, tag="xg", name="xg",
                                 bufs=4)
                    gi = nc.gpsimd.indirect_dma_start(
                        out=xg, out_offset=None, in_=recv_x[:],
                        in_offset=IndirectOffsetOnAxis(
                            ap=gin_sb[:, col:col + 1], axis=0),
                        bounds_check=p.xr_rows - 1, oob_is_err=False)
                    for h in range(p.cdep[ct] + 1):
                        bass._add_dep_helper(gi.ins, cc_disp[h].ins,
                                             sync=True,
                                             reason=f"gather after a2a{h}")
                    tiles.append(xg)
                xgs[ct] = tiles

            def emit_transpose(ct):
                """PE transposes (identity matmuls) feeding xrT, emitted just
                before the ctile's compute."""
                r0, NT = p.ctiles[ct]
                xrT = pC.tile([P, D_BLKS, N_CT], BF16, tag="xrT", name="xrT",
                              bufs=2)
                xrTs[ct] = xrT
                for cc in range(NT // P):
                    xg = xgs[ct][cc]
                    for j in range(D_BLKS):
                        tp = pCp.tile([P, P], BF16, tag="tp", name="tp",
                                      bufs=2)
                        nc.tensor.transpose(tp, xg[:, j * P:(j + 1) * P],
                                            ident_bf)
                        nc.vector.tensor_copy(
                            xrT[:, j, cc * P:(cc + 1) * P], tp)
                xgs.pop(ct)

            grp_scatters = [[] for _ in range(nG)]
            own_scatters = []
            last_scat = {}
            cc_rets = [None] * nG

            def emit_compute(ct, mid_cb=None):
                r0, NT = p.ctiles[ct]
                xrT = xrTs.pop(ct)
                hT = pC.tile([P, H_BLKS, N_CT], BF16, tag="hT", name="hT",
                             bufs=1)
                for m in range(H_BLKS):
                    if m == 6 and mid_cb is not None:
                        mid_cb()
                    ps1 = pCp.tile([P, N_CT], FP32, tag="ps1", name="ps1",
                                   bufs=3)
                    for j in range(D_BLKS):
                        nc.tensor.matmul(ps1[:, :NT],
                                         lhsT=w1_sb[:, j, m * P:(m + 1) * P],
                                         rhs=xrT[:, j, :NT],
                                         start=(j == 0),
                                         stop=(j == D_BLKS - 1))
                    nc.scalar.activation(hT[:, m, :NT], ps1[:, :NT], AF.Silu,
                                         bias=b1_sb[:, m:m + 1])

                for t in range(NT // P):
                    col = r0 // P + t
                    y_tm = pC.tile([P, D_MODEL], BF16, tag="y_tm",
                                   name="y_tm", bufs=3)
                    for nh in range(2):
                        ps2 = pCp.tile([P, 512], FP32, tag="ps2", name="ps2",
                                       bufs=3)
                        for m in range(H_BLKS):
                            nc.tensor.matmul(
                                ps2, lhsT=hT[:, m, t * P:(t + 1) * P],
                                rhs=w2_sb[:, m, nh * 512:(nh + 1) * 512],
                                start=(m == 0), stop=(m == H_BLKS - 1))
                        nc.vector.tensor_add(
                            y_tm[:, nh * 512:(nh + 1) * 512], ps2,
                            b2r_sb[:, nh * 512:(nh + 1) * 512])
                        nc.vector.tensor_scalar_mul(
                            y_tm[:, nh * 512:(nh + 1) * 512],
                            y_tm[:, nh * 512:(nh + 1) * 512],
                            wrow_sb[:, col:col + 1])
                    if ct < N_OWN_CT:
                        si = nc.gpsimd.indirect_dma_start(
                            out=rya[col].ap(),
                            out_offset=IndirectOffsetOnAxis(
                                ap=gout_sb[:, col:col + 1], axis=0),
                            in_=y_tm[:], in_offset=None,
                            bounds_check=p.yr_rows - 1, oob_is_err=False)
                        own_scatters.append(si)
                        last_scat[ct] = si
                    else:
                        si = nc.gpsimd.indirect_dma_start(
                            out=sya[col].ap(),
                            out_offset=IndirectOffsetOnAxis(
                                ap=gout_sb[:, col:col + 1], axis=0),
                            in_=y_tm[:], in_offset=None,
                            bounds_check=p.ys_rows - 1, oob_is_err=False)
                        grp_scatters[ct - N_OWN_CT].append(si)
                        last_scat[ct] = si

            def emit_ret(g):
                lo = p.ys_base[g]
                hi = lo + N_CORES * p.crs[g]
                cc = nc.gpsimd.collective_compute(
                    "AllToAll", ALU.bypass, replica_groups=RG,
                    ins=[send_y[lo:hi, :].opt()],
                    outs=[rya[OWN // P + g].ap()[lo:hi, :].opt()])
                for si in grp_scatters[g]:
                    bass._add_dep_helper(cc.ins, si.ins, sync=True,
                                         reason=f"ret a2a{g} after scatters")
                cc_rets[g] = cc

            def emit_E(tiles, pos_dep):
                """Combine pass for owner token tiles. The own-expert result
                rows are contiguous (own row index == local token index), so
                they arrive by direct DMA on the sync ring; only the net row
                needs an indirect gather with deps on exactly the return
                groups this tile's tokens ride. pos_dep pins the ops late
                enough in every engine stream that they can never block
                ctile work."""
                for i in tiles:
                    g0 = pC.tile([P, D_MODEL], BF16, tag="xg", name="g0",
                                 bufs=4)
                    di = nc.sync.dma_start(
                        g0, recv_y[p.yr_own + i * P:p.yr_own + (i + 1) * P,
                                   :])
                    g1 = pC.tile([P, D_MODEL], BF16, tag="xg", name="g1",
                                 bufs=4)
                    gi1 = nc.gpsimd.indirect_dma_start(
                        out=g1, out_offset=None, in_=recv_y[:],
                        in_offset=IndirectOffsetOnAxis(
                            ap=gres_sb[:, i:i + 1], axis=0),
                        bounds_check=p.yr_rows - 1, oob_is_err=False)
                    for gi in (di, gi1):
                        for si in own_scatters:
                            bass._add_dep_helper(gi.ins, si.ins, sync=True,
                                                 reason="E after own scat")
                        if pos_dep is not None:
                            bass._add_dep_helper(gi.ins, pos_dep.ins,
                                                 sync=True,
                                                 reason="E position pin")
                    for g in range(p.gmax[i] + 1):
                        bass._add_dep_helper(gi1.ins, cc_rets[g].ins,
                                             sync=True,
                                             reason="E after ret a2a")
                    acc = pE.tile([P, D_MODEL], FP32, tag="acc", name="acc",
                                  bufs=2)
                    nc.vector.tensor_add(acc, g0, g1)
                    nc.sync.dma_start(out_loc[i * P:(i + 1) * P, :], acc)

            def emit_patch(lastg, pos_dep):
                # re-gather + re-combine + partial-store only the tokens
                # whose result rows live in the final return group
                for ii in range(N_TOK_TILES):
                    if not p.p2_mask[ii]:
                        continue
                    g0 = pC.tile([P, D_MODEL], BF16, tag="xg", name="p2g0",
                                 bufs=4)
                    gi0 = nc.gpsimd.indirect_dma_start(
                        out=g0, out_offset=None, in_=recv_y[:],
                        in_offset=IndirectOffsetOnAxis(
                            ap=gp2_sb[:, ii, 0:1], axis=0),
                        bounds_check=p.yr_rows - 1, oob_is_err=False)
                    g1 = pC.tile([P, D_MODEL], BF16, tag="xg", name="p2g1",
                                 bufs=4)
                    gi1 = nc.gpsimd.indirect_dma_start(
                        out=g1, out_offset=None, in_=recv_y[:],
                        in_offset=IndirectOffsetOnAxis(
                            ap=gp2_sb[:, ii, 1:2], axis=0),
                        bounds_check=p.yr_rows - 1, oob_is_err=False)
                    for gi in (gi0, gi1):
                        bass._add_dep_helper(gi.ins, cc_rets[lastg].ins,
                                             sync=True,
                                             reason="E p2 after last ret")
                        if pos_dep is not None:
                            bass._add_dep_helper(gi.ins, pos_dep.ins,
                                                 sync=True,
                                                 reason="E p2 position pin")
                    acc = pE.tile([P, D_MODEL], FP32, tag="acc", name="acc",
                                  bufs=2)
                    nc.vector.tensor_add(acc, g0, g1)
                    nc.gpsimd.indirect_dma_start(
                        out=out_loc,
                        out_offset=IndirectOffsetOnAxis(
                            ap=op2_sb[:, ii:ii + 1], axis=0),
                        in_=acc, in_offset=None,
                        bounds_check=T_LOC - 1, oob_is_err=False)

            def mid_tables():
                # tables needed from mid-ctile-0 onward: issued on the
                # gpsimd ring (idle once the dispatch cascade is out) so
                # they never delay the SILUs on the scalar ring
                nc.gpsimd.dma_start(gout_sb, gout_t[:])
                nc.gpsimd.dma_start(b2r_sb, b2_rep[:])
                emit_own_load(1, eng=nc.gpsimd)
                nc.gpsimd.dma_start(gin_sb, gidx_in[:])
                nc.gpsimd.dma_start(gres_sb, gres_t[:])
                nc.gpsimd.dma_start(wrow_sb, wts_t[:])
                nc.gpsimd.dma_start(gp2_sb, gp2_t[:])
                nc.gpsimd.dma_start(op2_sb, op2_t[:])

            # per-tile combine positions: one ctile after the tile's last
            # return group is triggered, so a slow return can never block
            # a later ctile's scatters through the gpsimd FIFO
            pw = [N_OWN_CT + p.gmax[i] + 1 for i in range(N_TOK_TILES)]
            io_done = N_OWN_CT
            for ct in range(p.nc_tiles):
                if ct >= N_OWN_CT:
                    emit_transpose(ct)
                emit_compute(ct, mid_cb=mid_tables if ct == 0 else None)
                if ct >= N_OWN_CT:
                    emit_ret(ct - N_OWN_CT)
                while io_done < p.nc_tiles and io_done <= ct + 2:
                    emit_gather(io_done)
                    io_done += 1
                tiles_now = [i for i in range(N_TOK_TILES) if pw[i] == ct]
                if tiles_now:
                    emit_E(tiles_now, pos_dep=last_scat[ct])
            lastc = p.nc_tiles - 1
            late = [i for i in range(N_TOK_TILES) if pw[i] > lastc]
            if late:
                emit_E(late, pos_dep=last_scat[lastc])
            emit_patch(nG - 1, pos_dep=last_scat[lastc])


def build_kernel(plan):
    nc = bacc.Bacc("TRN2", target_bir_lowering=False, debug=False,
                   num_devices=N_CORES)
    NCOL = plan.s_all // P
    args = dict(
        x_bf=nc.dram_tensor("x_bf", [T_LOC, D_MODEL], BF16,
                            kind="ExternalInput"),
        xT_own=nc.dram_tensor("xT_own", [P, N_OWN_CT, D_BLKS, N_CT], BF16,
                              kind="ExternalInput"),
        rows_net=nc.dram_tensor("rows_net", [P, N_TOK_TILES, TOP_K], I32,
                                kind="ExternalInput"),
        w1_loc=nc.dram_tensor("w1_loc", [D_MODEL, HIDDEN], BF16,
                              kind="ExternalInput"),
        w2_loc=nc.dram_tensor("w2_loc", [HIDDEN, D_MODEL], BF16,
                              kind="ExternalInput"),
        b1_t=nc.dram_tensor("b1_t", [P, H_BLKS], FP32, kind="ExternalInput"),
        b2_rep=nc.dram_tensor("b2_rep", [P, D_MODEL], FP32,
                              kind="ExternalInput"),
        gidx_in=nc.dram_tensor("gidx_in", [P, NCOL], I32,
                               kind="ExternalInput"),
        gout_t=nc.dram_tensor("gout_t", [P, NCOL], I32,
                              kind="ExternalInput"),
        gres_t=nc.dram_tensor("gres_t", [P, N_TOK_TILES], I32,
                              kind="ExternalInput"),
        wts_t=nc.dram_tensor("wts_t", [P, NCOL], FP32,
                             kind="ExternalInput"),
        gp2_t=nc.dram_tensor("gp2_t", [P, N_TOK_TILES, TOP_K], I32,
                             kind="ExternalInput"),
        op2_t=nc.dram_tensor("op2_t", [P, N_TOK_TILES], I32,
                             kind="ExternalInput"),
        out_loc=nc.dram_tensor("out_loc", [T_LOC, D_MODEL], FP32,
                               kind="ExternalOutput"),
    )
    with tile.TileContext(nc) as tc:
        _body(tc, plan, **{k: v.ap() for k, v in args.items()})
    nc.compile()
    return nc


def _round_up(v, m):
    return ((v + m - 1) // m) * m


def _assign_tokens(top2):
    """Token->core matching: place each token on a core owning one of its
    two experts (augmenting paths fix greedy failures; leftovers spill to
    any core with space)."""
    cap = np.full(N_CORES, T_LOC, np.int64)
    assign = np.full(T, -1, np.int64)
    members = [[] for _ in range(N_CORES)]   # tokens assigned per core
    flow = np.zeros((N_CORES, N_CORES), np.int64)   # src -> net dst counts

    def place(t, c):
        assign[t] = c
        members[c].append(t)
        cap[c] -= 1
        for e in top2[t]:
            if int(e) != c:
                flow[c, int(e)] += 1

    deferred = []
    for t in range(T):
        # prefer the expert-core that keeps the directed pair flows
        # balanced (smaller send-count for this pair orientation)
        opts = [int(top2[t, k]) for k in range(TOP_K)]
        a, b = opts[0], opts[1]
        order_ = opts if flow[a, b] <= flow[b, a] else opts[::-1]
        for c in order_:
            if cap[c] > 0:
                place(t, c)
                break
        else:
            deferred.append(t)

    spills = []
    for t in deferred:
        # augment: find u on a full core c in top2(t) that can move to its
        # other expert's core with free capacity
        done = False
        for k in range(TOP_K):
            c = int(top2[t, k])
            for u in members[c]:
                alts = [int(e) for e in top2[u] if e != c]
                for d in alts:
                    if cap[d] > 0:
                        members[c].remove(u)
                        assign[u] = -1
                        cap[c] += 1
                        for e in top2[u]:
                            if int(e) != c:
                                flow[c, int(e)] -= 1
                        place(u, d)
                        place(t, c)
                        done = True
                        break
                if done:
                    break
            if done:
                break
        if not done:
            spills.append(t)
    for t in spills:
        c = int(np.argmax(cap))
        place(t, c)
    assert (cap == 0).all() and (assign >= 0).all()
    # interleave each core's tokens round-robin by net destination so the
    # (src,dst) flow counts are uniform across dispatch chunks and owner
    # counts are uniform across return groups
    perm = []
    for c in range(N_CORES):
        buckets = [[] for _ in range(N_CORES)]
        for t in sorted(members[c]):
            dsts = [int(e) for e in top2[t] if e != c]
            buckets[dsts[0]].append(t)
        keyed = []
        for bi, b in enumerate(buckets):
            for k, t in enumerate(b):
                keyed.append(((k + 0.5) / len(b), bi, t))
        order = [t for _, _, t in sorted(keyed)]
        perm.append(np.array(order, np.int64))
    return perm


def _make_plan_and_tables(flat_x, gate_w, gate_b):
    """Exact host routing + schedule. Returns (plan, per-core tables, perm)."""
    x64 = flat_x.astype(np.float64)
    logits = x64 @ gate_w.astype(np.float64) + gate_b.astype(np.float64)
    order = np.argsort(-logits, axis=1, kind="stable")
    top2 = order[:, :TOP_K]                       # [T, 2]
    l0 = np.take_along_axis(logits, top2, axis=1)
    w0 = 1.0 / (1.0 + np.exp(-(l0[:, 0] - l0[:, 1])))
    wts = np.stack([w0, 1.0 - w0], axis=1).astype(np.float32)  # [T, 2]

    perm = _assign_tokens(top2)
    # per-core views in local token order
    core_of = np.empty(T, np.int64)
    loc_of = np.empty(T, np.int64)
    for c in range(N_CORES):
        core_of[perm[c]] = c
        loc_of[perm[c]] = np.arange(T_LOC)

    # own rows: the (at most one) expert matching the assigned core.
    # own row index == local token index.
    own_k = np.full(T, -1, np.int64)
    for t in range(T):
        for k in range(TOP_K):
            if top2[t, k] == core_of[t]:
                own_k[t] = k
                break

    # --- network slots per dispatch chunk (chunk = local tile pair) ---
    cnt_net = np.zeros((NCH, N_CORES, N_CORES), np.int64)  # [chunk, src, dst]
    net_slot = np.full((T, TOP_K), -1, np.int64)
    for t in range(T):
        s = core_of[t]
        h = loc_of[t] // (P * 2)
        for k in range(TOP_K):
            if k == own_k[t]:
                continue
            d = int(top2[t, k])
            net_slot[t, k] = cnt_net[h, s, d]
            cnt_net[h, s, d] += 1
    caps = [int(_round_up(max(1, cnt_net[h].max()), 16)) for h in range(NCH)]

    # --- per-core compute rows ---
    n_net_rows = cnt_net.sum(axis=1)              # [chunk, dst]
    per_core_rows = OWN + n_net_rows.sum(axis=0)
    s_all = int(_round_up(per_core_rows.max(), P))

    ctl = _ctile_sizes(s_all)
    ends = [off + nt for off, nt in ctl]
    n_ctiles = len(ctl)
    nG = n_ctiles - N_OWN_CT
    avail = [OWN + int(n_net_rows[:h + 1].sum(axis=0).min())
             for h in range(NCH)]
    cdep = []
    for ct, e in enumerate(ends):
        if e <= OWN:
            cdep.append(-1)
        else:
            cdep.append(next((h for h in range(NCH) if e <= avail[h]),
                             NCH - 1))

    row_to_ct = np.zeros(s_all, np.int64)
    for ct, (off, nt) in enumerate(ctl):
        row_to_ct[off:off + nt] = ct

    probe = Plan(caps, s_all, cdep, [16] * nG, (0,) * N_TOK_TILES)
    XNET = probe.x_base
    XSCR = probe.xr_scratch

    gin = np.zeros((N_CORES, s_all), np.int64)
    row_kind = np.zeros((N_CORES, s_all), np.int8)   # 0 pad, 1 own, 2 net
    row_owner = np.zeros((N_CORES, s_all), np.int64)
    net_comp_row = np.zeros((NCH, N_CORES, N_CORES, max(caps)), np.int64)
    for c in range(N_CORES):
        row_kind[c, :OWN] = 1
        row_owner[c, :OWN] = c
        r = OWN
        for h in range(NCH):
            # round-robin across srcs: every ctile's result rows spread
            # evenly over owners -> small per-group return capacities
            hi = int(cnt_net[h, :, c].max())
            for i in range(hi):
                for s in range(N_CORES):
                    if i < int(cnt_net[h, s, c]):
                        gin[c, r] = XNET[h] + s * caps[h] + i
                        row_kind[c, r] = 2
                        row_owner[c, r] = s
                        net_comp_row[h, s, c, i] = r
                        r += 1
        while r < s_all:
            gin[c, r] = XSCR + (r % P)
            r += 1

    # per-compute-row routing weights (applied expert-side to y rows)
    wrow = np.zeros((N_CORES, s_all), np.float32)
    for t in range(T):
        s = core_of[t]
        for k in range(TOP_K):
            if k == own_k[t]:
                wrow[s][loc_of[t]] = wts[t, k]
            else:
                d = int(top2[t, k])
                h = loc_of[t] // (P * 2)
                r = int(net_comp_row[h, s, d, int(net_slot[t, k])])
                wrow[d][r] = wts[t, k]

    # per-ctile return groups
    crs_count = np.zeros((N_CORES, nG, N_CORES), np.int64)
    ret_idx = np.zeros((N_CORES, s_all), np.int64)
    ret_grp = np.full((N_CORES, s_all), -1, np.int64)
    for c in range(N_CORES):
        for r in range(OWN, s_all):
            if row_kind[c, r] != 2:
                continue
            g = int(row_to_ct[r]) - N_OWN_CT
            o = row_owner[c, r]
            ret_grp[c, r] = g
            ret_idx[c, r] = crs_count[c, g, o]
            crs_count[c, g, o] += 1
    crs = [int(_round_up(max(1, crs_count[:, g, :].max()), 8))
           for g in range(nG)]

    plan = Plan(caps, s_all, cdep, crs, (0,) * N_TOK_TILES)

    # --- device tables per core ---
    NCOL = s_all // P
    tabs = []
    lastg = nG - 1
    p2_any = np.zeros(N_TOK_TILES, bool)
    gmax = np.zeros(N_TOK_TILES, np.int64)
    x_bf_full = flat_x.astype(BF16_NP)
    for c in range(N_CORES):
        x_core = np.ascontiguousarray(x_bf_full[perm[c]])   # [T_LOC, D]
        rows_net_t = np.full((P, N_TOK_TILES, TOP_K), OOB, np.int32)
        gres = np.zeros((P, N_TOK_TILES), np.int32)
        gp2 = np.full((P, N_TOK_TILES, TOP_K), OOB, np.int32)
        op2 = np.full((P, N_TOK_TILES), OOB, np.int32)
        for tl in range(T_LOC):
            t = int(perm[c][tl])
            i, pp = tl // P, tl % P
            h = tl // (P * 2)
            nets = []
            for k in range(TOP_K):
                if k == own_k[t]:
                    continue
                d = int(top2[t, k])
                idx = int(net_slot[t, k])
                rows_net_t[pp, i, k] = XNET[h] + d * caps[h] + idx
                r = int(net_comp_row[h, c, d, idx])
                g = int(ret_grp[d, r])
                nets.append((plan.yr_base[g] + d * crs[g]
                             + int(ret_idx[d, r]), g))
            gres[pp, i] = nets[0][0]
            gmax[i] = max(gmax[i], nets[0][1])
            if len(nets) > 1:
                # spilled token (no own row): the tile's combine adds a
                # zero own row + nets[0]; the patch rewrites with both
                # net rows after the final return
                p2_any[i] = True
                gp2[pp, i, 0] = nets[0][0]
                gp2[pp, i, 1] = nets[1][0]
                op2[pp, i] = tl

        gout = np.zeros((P, NCOL), np.int32)
        for r in range(s_all):
            col, pp = r // P, r % P
            if row_kind[c, r] == 1:
                gout[pp, col] = plan.yr_own + r   # own row j == r
            elif row_kind[c, r] == 2:
                g = int(ret_grp[c, r])
                o = int(row_owner[c, r])
                gout[pp, col] = (plan.ys_base[g] + o * crs[g]
                                 + int(ret_idx[c, r]))
            else:
                gout[pp, col] = plan.ys_scratch + pp
        gin32 = np.ascontiguousarray(
            gin[c].reshape(NCOL, P).T.astype(np.int32))
        gout = np.ascontiguousarray(gout)
        wrow32 = np.ascontiguousarray(wrow[c].reshape(NCOL, P).T)

        # whole per-core x pre-transposed, packed per own-ctile
        xT_own = np.zeros((P, N_OWN_CT, D_BLKS, N_CT), BF16_NP)
        A = x_core.reshape(N_OWN_CT, N_CT, D_BLKS, P)
        xT_own[:, :, :, :] = A.transpose(3, 0, 2, 1)

        tabs.append(dict(gidx_in=gin32, gout_t=gout, gres_t=gres,
                         wts_t=wrow32, gp2_t=gp2, op2_t=op2,
                         rows_net=rows_net_t,
                         x_bf=x_core,
                         xT_own=np.ascontiguousarray(xT_own)))
    plan.p2_mask = tuple(bool(x) for x in p2_any)
    plan.gmax = tuple(int(g) for g in gmax)
    return plan, tabs, perm


_CACHE = {}


def kernel(x, gate_w, gate_b, w1, b1, w2, b2, _trace=False):
    x = np.ascontiguousarray(np.asarray(x, dtype=np.float32))
    gate_w = np.ascontiguousarray(np.asarray(gate_w, dtype=np.float32))
    gate_b = np.ascontiguousarray(np.asarray(gate_b, dtype=np.float32))
    w1 = np.ascontiguousarray(np.asarray(w1, dtype=np.float32))
    b1 = np.ascontiguousarray(np.asarray(b1, dtype=np.float32))
    w2 = np.ascontiguousarray(np.asarray(w2, dtype=np.float32))
    b2 = np.ascontiguousarray(np.asarray(b2, dtype=np.float32))

    orig_shape = x.shape
    flat_x = x.reshape(-1, D_MODEL)
    plan, tabs, perm = _make_plan_and_tables(flat_x, gate_w, gate_b)

    if plan.key() not in _CACHE:
        _CACHE[plan.key()] = build_kernel(plan)
    nc = _CACHE[plan.key()]

    in_maps = []
    for c in range(N_CORES):
        m = dict(tabs[c])
        m["w1_loc"] = np.ascontiguousarray(w1[c].astype(BF16_NP))
        m["w2_loc"] = np.ascontiguousarray(w2[c].astype(BF16_NP))
        m["b1_t"] = np.ascontiguousarray(b1[c].reshape(H_BLKS, P).T)
        m["b2_rep"] = np.tile(b2[c], (P, 1))
        in_maps.append(m)

    res = run_bass_kernel_spmd(nc, in_maps, core_ids=list(range(N_CORES)),
                               trace=_trace)
    out = np.empty((T, D_MODEL), np.float32)
    for c in range(N_CORES):
        out[perm[c]] = res.results[c]["out_loc"]
    if _trace:
        kernel.last_results = res
    return out.reshape(orig_shape)
